# revision 1
# baseline (speedup 1.0000x reference)
"""GQA attention kernel for 8 Trainium2 NeuronCores — v2.

Sharding: core c = 4*b + h handles batch b (of 2) and kv-head h (of 4),
i.e. one kv head + its 4 grouped query heads. Each core computes its head
group's partial contribution to the output projection; the host sums the
4 partials per batch. No collectives.

v2 changes vs v1 (461us):
- all matmul inputs bf16 (x, Wq/k/v already Wo) — halves DMA traffic;
  bf16 matmul is 1 cyc/row like f32r, but ldweights are cheaper
- softmax denominator: accumulate the exp tiles into R via two parallel
  chains (GpSimd: even sk-pairs, DVE: odd sk-pairs), fold, then ONE
  512-row matmul with an all-ones [128,128] stationary per (qtile, head)
  — the PE both reduces over partitions AND replicates den across all
  128 output partitions, so no gpsimd broadcast is needed. Replaces the
  per-sk ones-matmuls that were 1/3 of attention-phase PE rows.
- reciprocal_approx_fast instead of reciprocal (~5x faster)
- software pipelining: outproj(qt-1) i-blocks interleaved between
  attn(qt) head-groups — pure-PE outproj stretches let the scalar
  engine (exp is 1.11us/sk-pair vs PE's 0.85us) catch up; scores
  emitted one sk-pair ahead of AV; normalize deferred one head-group
- BAND 256->512: ap-512 projection matmuls

Device math per core (S=2048, H=2048, d=128):
  QT_g = (x @ Wq_g + bq_g)^T          [d, S]   g=0..3   (bf16)
  KT   = (x @ Wk_h)^T                 [d, S]            (bk cancels in softmax)
  V    = x @ Wv_h                     [S, d]   (computed as V^T then PE-transposed)
  S^T  = KT^T-blocks @ QT             [Sk, Sq]
  P^T  = exp(SCALE * S^T)             (bf16, no max-subtraction: |s| <~ 6)
  den  = ones.T @ (DVE-sum of P^T tiles)
  y^T  = V^T-blocks.T @ P^T (PSUM accum);  yT := y^T * (1/den)
  out += yT_g^T @ Wo_g                [S, H]  (partial over this head group)
Host: out[b] = sum_h partial + (bv_rep @ Wo + bo).
"""

import numpy as np
import ml_dtypes

B = 2
S = 2048
HIDDEN = 2048
NKV = 4
GROUP = 4
D = 128
SCALE = D ** -0.5

BAND = 512            # S-columns per projection band
NBAND = S // BAND     # 4
NCH = HIDDEN // 128   # 16 contraction chunks
QTILE = 512           # queries per attention tile
NQT = S // QTILE      # 4
NSK = S // 128        # 16 key tiles
NSKP = NSK // 2       # 8 sk pairs

_CACHE = {}
LAST_RESULTS = None
TRACE = False
TMPDIR = None


def _build():
    import concourse.bass as bass
    import concourse.bacc as bacc
    import concourse.mybir as mybir
    import concourse.tile as tile
    from concourse.masks import make_identity

    f32 = mybir.dt.float32
    bf16 = mybir.dt.bfloat16
    EXP = mybir.ActivationFunctionType.Exp
    IDENT = mybir.ActivationFunctionType.Identity
    COPY = mybir.ActivationFunctionType.Copy

    nc = bacc.Bacc(trn_type="TRN2", target_bir_lowering=False, debug=False)

    xT = nc.dram_tensor("xT", [NBAND, 128, NCH, BAND], bf16, kind="ExternalInput").ap()
    wq = nc.dram_tensor("wq", [GROUP, 128, NCH, 128], bf16, kind="ExternalInput").ap()
    wk = nc.dram_tensor("wk", [128, NCH, 128], bf16, kind="ExternalInput").ap()
    wv = nc.dram_tensor("wv", [128, NCH, 128], bf16, kind="ExternalInput").ap()
    wo = nc.dram_tensor("wo", [GROUP, 128, HIDDEN], bf16, kind="ExternalInput").ap()
    bq = nc.dram_tensor("bq", [128, GROUP], f32, kind="ExternalInput").ap()
    onesk = nc.dram_tensor("onesk", [128, 128], bf16, kind="ExternalInput").ap()
    out = nc.dram_tensor("out", [S, HIDDEN], f32, kind="ExternalOutput").ap()

    with tile.TileContext(nc) as tc:
        with (
            tc.tile_pool(name="const", bufs=1) as constp,
            tc.tile_pool(name="wts", bufs=1) as wtsp,
            tc.tile_pool(name="xb", bufs=2) as xbp,
            tc.tile_pool(name="qkv", bufs=1) as qkvp,
            tc.tile_pool(name="ptbuf", bufs=2) as ptp,
            tc.tile_pool(name="rbuf", bufs=2) as rp,
            tc.tile_pool(name="dens", bufs=2) as densp,
            tc.tile_pool(name="ytbuf", bufs=8) as ytp,
            tc.tile_pool(name="outbuf", bufs=2) as outp,
        ):
            # ---- DMAs in consumption order ----
            onesk_t = constp.tile([128, 128], bf16, name="onesk_t")
            nc.sync.dma_start(out=onesk_t[:, :], in_=onesk)
            bq_t = constp.tile([128, GROUP], f32, name="bq_t")
            nc.sync.dma_start(out=bq_t[:, :], in_=bq)
            ident = constp.tile([128, 128], f32, name="ident")
            make_identity(nc, ident[:, :])

            wk_t = wtsp.tile([128, NCH, 128], bf16, name="wk_t")
            nc.sync.dma_start(out=wk_t[:, :, :], in_=wk)

            # band 0 split into 4 quarter-DMAs (separate tiles) so the first
            # K-projection chunk matmuls start as soon as the first quarter
            # lands instead of waiting for the whole 2.1 MB band
            bands = [None] * NBAND
            b0q = []
            for c4 in range(4):
                t = xbp.tile([128, 4, BAND], bf16, name=f"b0q{c4}", tag="band0q",
                             bufs=4)
                nc.sync.dma_start(out=t[:, :, :], in_=xT[0, :, 4 * c4:4 * c4 + 4, :])
                b0q.append(t)

            wv_t = wtsp.tile([128, NCH, 128], bf16, name="wv_t")
            nc.sync.dma_start(out=wv_t[:, :, :], in_=wv)
            wq_t = []
            for g in range(GROUP):
                t = wtsp.tile([128, NCH, 128], bf16, name=f"wq_t{g}", tag=f"wq{g}")
                nc.sync.dma_start(out=t[:, :, :], in_=wq[g])
                wq_t.append(t)
            bands[1] = xbp.tile([128, NCH, BAND], bf16, name="band", tag="band")
            nc.sync.dma_start(out=bands[1][:, :, :], in_=xT[1])
            wo_t = []
            for g in range(GROUP):
                t = wtsp.tile([128, HIDDEN], bf16, name=f"wo_t{g}", tag=f"wo{g}")
                nc.sync.dma_start(out=t[:, :], in_=wo[g])
                wo_t.append(t)

            # ---- persistent activations ----
            qt_t = []
            for g in range(GROUP):
                t = qkvp.tile([128, S], bf16, name=f"qt{g}", tag=f"qt{g}")
                qt_t.append(t)
            kt_t = qkvp.tile([128, S], bf16, name="kt_t")
            v_t = qkvp.tile([128, S], bf16, name="v_t")
            vt_f = qkvp.tile([128, S], f32, name="vt_f")

            # =============== phase 1: projections ===============
            with tc.tile_pool(name="psA", bufs=1, space="PSUM") as psA:
                for bd in range(NBAND):
                    if bd > 0 and bands[bd] is None:
                        bands[bd] = xbp.tile(
                            [128, NCH, BAND], bf16, name="band", tag="band"
                        )
                        nc.sync.dma_start(out=bands[bd][:, :, :], in_=xT[bd])

                    def bch(c, bd=bd):
                        if bd == 0:
                            return b0q[c // 4][:, c % 4, :]
                        return bands[bd][:, c, :]

                    bsl = slice(bd * BAND, (bd + 1) * BAND)

                    # K^T accumulation
                    pk = psA.tile([128, BAND], f32, name="pk", tag="pacc", bufs=3)
                    for c in range(NCH):
                        nc.tensor.matmul(
                            out=pk[:, :],
                            lhsT=wk_t[:, c, :],
                            rhs=bch(c),
                            start=(c == 0), stop=(c == NCH - 1),
                        )
                    nc.scalar.activation(kt_t[:, bsl], pk[:, :], COPY)

                    # V^T accumulation (f32, transposed to V per 128-block later)
                    pv = psA.tile([128, BAND], f32, name="pv", tag="pacc", bufs=3)
                    for c in range(NCH):
                        nc.tensor.matmul(
                            out=pv[:, :],
                            lhsT=wv_t[:, c, :],
                            rhs=bch(c),
                            start=(c == 0), stop=(c == NCH - 1),
                        )
                    nc.scalar.activation(vt_f[:, bsl], pv[:, :], COPY)

                    # Q^T per local head
                    for g in range(GROUP):
                        pq = psA.tile([128, BAND], f32, name="pq", tag="pacc", bufs=3)
                        for c in range(NCH):
                            nc.tensor.matmul(
                                out=pq[:, :],
                                lhsT=wq_t[g][:, c, :],
                                rhs=bch(c),
                                start=(c == 0), stop=(c == NCH - 1),
                            )
                        nc.scalar.activation(
                            qt_t[g][:, bsl], pq[:, :], IDENT,
                            bias=bq_t[:, g:g + 1],
                        )

                    # transpose V^T band -> V (BAND//128 sk-tiles per band)
                    for t in range(BAND // 128):
                        sk = bd * (BAND // 128) + t
                        pt = psA.tile([128, 128], f32, name="ptr", tag="pacc", bufs=3)
                        nc.tensor.transpose(
                            pt[:, :], vt_f[:, sk * 128:(sk + 1) * 128], ident[:, :]
                        )
                        nc.scalar.activation(
                            v_t[:, sk * 128:(sk + 1) * 128], pt[:, :], COPY
                        )

            # =============== phase 2+3: attention + out-projection ===============
            with tc.tile_pool(name="psB", bufs=1, space="PSUM") as psB:
                yt_all = {}
                pending = [None]  # deferred normalize closure

                def flush_pending():
                    if pending[0] is not None:
                        pending[0]()
                        pending[0] = None

                def attn_group(qt, g, proj_qt=None, proj_i=None):
                    """One head-group of attention over qtile qt. When
                    proj_qt/proj_i are given, the 16 out-projection matmuls
                    for block (proj_qt, proj_i) are interleaved one per sk
                    iteration, filling the PE while it would otherwise wait
                    on the scalar engine's exp (0.59us/sk vs 0.43us of
                    attention-only PE work per sk)."""
                    qsl = slice(qt * QTILE, (qt + 1) * QTILE)
                    py = psB.tile([128, QTILE], f32, name="py", tag="yacc", bufs=2)
                    pt_all = ptp.tile(
                        [128, NSK * QTILE], bf16, name="pt", tag="pt"
                    )

                    def pt(a, b):
                        return pt_all[:, a * QTILE:b * QTILE]

                    # DVE accumulation chain in bf16 (2x DVE rate; positive
                    # sums, so the rounding error averages out in den)
                    R = rp.tile([128, QTILE], bf16, name="racc", tag="racc")
                    outs = po = ps2 = None
                    pair_mode = proj_qt is None
                    if proj_qt is not None:
                        outs = outp.tile([128, HIDDEN], f32, name="outs", tag="outs")

                    for sk in range(NSK):
                        if pair_mode:
                            # no outproj to interleave (qt 0): halve the
                            # scalar exp instruction count with paired-sk
                            # score tiles in the idle "po" psum space —
                            # the scalar engine is the floor here
                            if sk % 2 == 0:
                                ps2 = psB.tile(
                                    [128, 2 * QTILE], f32, name="ps2",
                                    tag="po", bufs=1,
                                )
                            h = sk % 2
                            nc.tensor.matmul(
                                out=ps2[:, h * QTILE:(h + 1) * QTILE],
                                lhsT=kt_t[:, sk * 128:(sk + 1) * 128],
                                rhs=qt_t[g][:, qsl],
                                start=True, stop=True,
                            )
                            if h == 1:
                                nc.scalar.activation(
                                    pt(sk - 1, sk + 1), ps2[:, :], EXP,
                                    scale=SCALE,
                                )
                                if sk == 1:
                                    nc.vector.tensor_add(
                                        R[:, :], pt(0, 1), pt(1, 2)
                                    )
                                else:
                                    nc.vector.tensor_add(
                                        R[:, :], R[:, :], pt(sk - 1, sk)
                                    )
                                    nc.vector.tensor_add(
                                        R[:, :], R[:, :], pt(sk, sk + 1)
                                    )
                        else:
                            ps = psB.tile(
                                [128, QTILE], f32, name="ps", tag="sc", bufs=3
                            )
                            nc.tensor.matmul(
                                out=ps[:, :],
                                lhsT=kt_t[:, sk * 128:(sk + 1) * 128],
                                rhs=qt_t[g][:, qsl],
                                start=True, stop=True,
                            )
                            nc.scalar.activation(
                                pt(sk, sk + 1), ps[:, :], EXP, scale=SCALE
                            )
                            if sk == 1:
                                nc.vector.tensor_add(
                                    R[:, :], pt(0, 1), pt(1, 2)
                                )
                            elif sk >= 2:
                                nc.vector.tensor_add(
                                    R[:, :], R[:, :], pt(sk, sk + 1)
                                )
                        if sk >= 2:
                            # AV trails scores by two sk so exp latency and
                            # jitter stay hidden
                            psk = sk - 2
                            nc.tensor.matmul(
                                out=py[:, :],
                                lhsT=v_t[:, psk * 128:(psk + 1) * 128],
                                rhs=pt(psk, psk + 1),
                                start=(psk == 0), stop=False,
                            )
                        if proj_qt is not None:
                            # out-projection: po half a (cols 0:1024) over
                            # sk 0-7, half b (cols 1024:2048) over sk 8-15;
                            # each half accumulates j two column-512 groups
                            # over the 4 heads
                            if sk % 8 == 0:
                                po = psB.tile(
                                    [128, 2 * QTILE], f32, name="po",
                                    tag="po", bufs=1,
                                )
                            jh = sk % 8   # (g', j) pair index within half
                            gp, j = jh // 2, 2 * (sk // 8) + jh % 2
                            nc.tensor.matmul(
                                out=po[:, (j % 2) * 512:(j % 2 + 1) * 512],
                                lhsT=yt_all[(proj_qt, gp)][
                                    :, proj_i * 128:(proj_i + 1) * 128],
                                rhs=wo_t[gp][:, j * 512:(j + 1) * 512],
                                start=(gp == 0), stop=(gp == GROUP - 1),
                            )
                            if sk % 8 == 7:
                                half = sk // 8
                                csl = slice(half * 1024, half * 1024 + 1024)
                                nc.vector.tensor_copy(outs[:, csl], po[:, :])
                        if sk == 2:
                            flush_pending()
                    for psk in (NSK - 2, NSK - 1):
                        nc.tensor.matmul(
                            out=py[:, :],
                            lhsT=v_t[:, psk * 128:(psk + 1) * 128],
                            rhs=pt(psk, psk + 1),
                            start=False, stop=(psk == NSK - 1),
                        )
                    if proj_qt is not None:
                        r0 = proj_qt * QTILE + proj_i * 128
                        nc.sync.dma_start(out=out[r0:r0 + 128, :], in_=outs[:, :])

                    def normalize(qt=qt, g=g, py=py, R=R):
                        # all-ones stationary: out[i, q] = sum_p R[p, q] —
                        # partition-reduces AND replicates den on all 128
                        # partitions in one 512-row matmul
                        pden = psB.tile(
                            [128, QTILE], f32, name="pden", tag="den", bufs=1
                        )
                        nc.tensor.matmul(
                            out=pden[:, :],
                            lhsT=onesk_t[:, :],
                            rhs=R[:, :],
                            start=True, stop=True,
                        )
                        rb = densp.tile([128, QTILE], f32, name="rb", tag="rb")
                        nc.vector.reciprocal_approx_fast(rb[:, :], pden[:, :])
                        yt = ytp.tile([128, QTILE], bf16, name="yt", tag="yt")
                        nc.vector.tensor_mul(yt[:, :], py[:, :], rb[:, :])
                        yt_all[(qt, g)] = yt

                    pending[0] = normalize

                def outproj_block_tail(qt, i):
                    # attention is done here, so the "sc" psum tag is free:
                    # rotate 512-col chunks through its 3 buffers so the PE
                    # never waits on the DVE psum->sbuf copies
                    outs = outp.tile([128, HIDDEN], f32, name="outs", tag="outs")
                    for j in range(4):
                        poc = psB.tile([128, QTILE], f32, name="poc",
                                       tag="sc", bufs=3)
                        for gp in range(GROUP):
                            nc.tensor.matmul(
                                out=poc[:, :],
                                lhsT=yt_all[(qt, gp)][:, i * 128:(i + 1) * 128],
                                rhs=wo_t[gp][:, j * 512:(j + 1) * 512],
                                start=(gp == 0), stop=(gp == GROUP - 1),
                            )
                        nc.vector.tensor_copy(
                            outs[:, j * 512:(j + 1) * 512], poc[:, :]
                        )
                    r0 = qt * QTILE + i * 128
                    nc.sync.dma_start(out=out[r0:r0 + 128, :], in_=outs[:, :])

                for qt in range(NQT):
                    for g in range(GROUP):
                        if qt >= 1:
                            attn_group(qt, g, proj_qt=qt - 1, proj_i=g)
                        else:
                            attn_group(qt, g)
                flush_pending()
                for i in range(QTILE // 128):
                    outproj_block_tail(NQT - 1, i)

    nc.finalize()
    return nc


def _get_nc():
    if "nc" not in _CACHE:
        _CACHE["nc"] = _build()
    return _CACHE["nc"]


def kernel(x, Wq, bq, Wk, bk, Wv, bv, Wo, bo):
    global LAST_RESULTS
    from concourse.bass_utils import run_bass_kernel_spmd

    bf = ml_dtypes.bfloat16
    x = np.asarray(x, np.float32)
    Wq = np.asarray(Wq, np.float32)
    Wk = np.asarray(Wk, np.float32)
    Wv = np.asarray(Wv, np.float32)
    Wo = np.asarray(Wo, np.float32)
    bq = np.asarray(bq, np.float32)
    bv = np.asarray(bv, np.float32)
    bo = np.asarray(bo, np.float32)

    nc = _get_nc()

    onesk_np = np.ones((128, 128), bf)

    in_maps = []
    for c in range(8):
        b, h = divmod(c, NKV)
        xTb = x[b].T  # [HIDDEN, S]
        xTh = np.ascontiguousarray(
            xTb.reshape(NCH, 128, NBAND, BAND).transpose(2, 1, 0, 3)
        ).astype(bf)
        # wq[g]: [128, NCH, 128] per local head
        wqh = np.ascontiguousarray(
            Wq[:, h * 512:(h + 1) * 512]
            .reshape(NCH, 128, GROUP, 128).transpose(2, 1, 0, 3)
        ).astype(bf)
        wkh = np.ascontiguousarray(
            Wk[:, h * 128:(h + 1) * 128].reshape(NCH, 128, 128).transpose(1, 0, 2)
        ).astype(bf)
        wvh = np.ascontiguousarray(
            Wv[:, h * 128:(h + 1) * 128].reshape(NCH, 128, 128).transpose(1, 0, 2)
        ).astype(bf)
        woh = np.ascontiguousarray(
            Wo[h * 512:(h + 1) * 512, :].reshape(GROUP, 128, HIDDEN)
        ).astype(bf)
        bqh = np.ascontiguousarray(
            bq[h * 512:(h + 1) * 512].reshape(GROUP, 128).T
        )
        in_maps.append({
            "xT": xTh, "wq": wqh, "wk": wkh, "wv": wvh, "wo": woh,
            "bq": bqh, "onesk": onesk_np,
        })

    res = run_bass_kernel_spmd(
        nc, in_maps, list(range(8)), trace=TRACE, tmpdir=TMPDIR
    )
    LAST_RESULTS = res

    # host-side constant bias: (bv repeated per head group) @ Wo + bo
    bv_rep = np.broadcast_to(
        bv.reshape(NKV, 1, D), (NKV, GROUP, D)
    ).reshape(HIDDEN)
    bias_row = bv_rep @ Wo + bo  # [HIDDEN]

    out = np.empty((B, S, HIDDEN), np.float32)
    for b in range(B):
        acc = res.results[b * NKV + 0]["out"].astype(np.float32)
        for h in range(1, NKV):
            acc = acc + res.results[b * NKV + h]["out"]
        out[b] = acc + bias_row
    return out



# revision 2
# speedup vs baseline: 1.1970x; 1.1970x over previous
"""GQA attention kernel for 8 Trainium2 NeuronCores — v3.

Sharding: core c = 4*b + h handles batch b (of 2) and kv-head h (of 4),
i.e. one kv head + its 4 grouped query heads. Each core computes its head
group's partial contribution to the output projection; the host sums the
4 partials per batch. No collectives.

v3 changes vs v2 (404us traced):
- The PE is the global bottleneck (~1200 N=512 matmuls x 259ns = ~311us
  at the observed 2.0 GHz sustained clock); every other engine has slack.
  So the entire schedule is rebuilt around "PE never idles":
- Q projections for S-bands 1-3 are DEFERRED out of the projection phase
  and interleaved as per-sk filler matmuls inside the attention passes
  (qt0 gets bands 1+2 = 2 filler MMs/sk, qt1 gets band 3). This kills
  v2's qt0 "pair mode", whose 1-buf psum pair serialized everything
  behind each EXP (PE duty ~30%, HAM-cold matmuls at 2x duration,
  ~42us lost).
- Out-projection of qt0/qt1 interleaves into qt2/qt3 (one j-chunk at a
  time through a 3-buf psum rotation); outproj of qt2/qt3 runs as a
  dense pure-PE tail. Total PE work is unchanged; only placed so no
  attention pass is EXP-bound.
- 12 warm-up matmuls on an identity tile (no DMA dependency) fill the
  ~9us DMA startup hole and get the PE past the HAM cold window before
  real data lands; wk is DMA'd in 4 chunk-quarters so the first real
  matmul starts on the first quarter.
- Output partials in bf16 (halves output DMA; host sums in f32);
  V^T transposed in bf16 (1 cyc/row instead of 2).
- Single attention-phase PSUM pool: sc x2 + py x2 + pden x1 + aux x3 = 8
  banks; aux serves qt0/qt1 Q-proj accumulators and qt2/qt3+tail
  outproj chunks through rotation.

Device math per core (S=2048, H=2048, d=128):
  QT_g = (x @ Wq_g + bq_g)^T          [d, S]   g=0..3   (bf16)
  KT   = (x @ Wk_h)^T                 [d, S]            (bk cancels in softmax)
  V    = x @ Wv_h                     [S, d]   (computed as V^T then PE-transposed)
  S^T  = KT^T-blocks @ QT             [Sk, Sq]
  P^T  = exp(SCALE * S^T)             (bf16, no max-subtraction: |s| <~ 6)
  den  = ones.T @ (DVE-sum of P^T tiles)
  y^T  = V^T-blocks.T @ P^T (PSUM accum);  yT := y^T * (1/den)
  out += yT_g^T @ Wo_g                [S, H]  (partial over this head group)
Host: out[b] = sum_h partial + (bv_rep @ Wo + bo).
"""

import numpy as np
import ml_dtypes

B = 2
S = 2048
HIDDEN = 2048
NKV = 4
GROUP = 4
D = 128
SCALE = D ** -0.5

BAND = 512            # S-columns per projection band
NBAND = S // BAND     # 4
NCH = HIDDEN // 128   # 16 contraction chunks
QTILE = 512           # queries per attention tile
NQT = S // QTILE      # 4
NSK = S // 128        # 16 key tiles
NWARM = 12            # PE warmup matmuls during DMA startup

_CACHE = {}
LAST_RESULTS = None
TRACE = False
TMPDIR = None


def _build():
    import concourse.bass as bass
    import concourse.bacc as bacc
    import concourse.mybir as mybir
    import concourse.tile as tile
    from concourse.masks import make_identity

    f32 = mybir.dt.float32
    bf16 = mybir.dt.bfloat16
    EXP = mybir.ActivationFunctionType.Exp
    IDENT = mybir.ActivationFunctionType.Identity
    COPY = mybir.ActivationFunctionType.Copy

    nc = bacc.Bacc(trn_type="TRN2", target_bir_lowering=False, debug=False)

    xT = nc.dram_tensor("xT", [NBAND, 128, NCH, BAND], bf16, kind="ExternalInput").ap()
    wq = nc.dram_tensor("wq", [GROUP, 128, NCH, 128], bf16, kind="ExternalInput").ap()
    wk = nc.dram_tensor("wk", [128, NCH, 128], bf16, kind="ExternalInput").ap()
    wv = nc.dram_tensor("wv", [128, NCH, 128], bf16, kind="ExternalInput").ap()
    wo = nc.dram_tensor("wo", [GROUP, 128, HIDDEN], bf16, kind="ExternalInput").ap()
    bq = nc.dram_tensor("bq", [128, GROUP], f32, kind="ExternalInput").ap()
    onesk = nc.dram_tensor("onesk", [128, 128], bf16, kind="ExternalInput").ap()
    out = nc.dram_tensor("out", [S, HIDDEN], bf16, kind="ExternalOutput").ap()

    with tile.TileContext(nc) as tc:
        with (
            tc.tile_pool(name="const", bufs=1) as constp,
            tc.tile_pool(name="wts", bufs=1) as wtsp,
            tc.tile_pool(name="xb", bufs=1) as xbp,
            tc.tile_pool(name="qkv", bufs=1) as qkvp,
            tc.tile_pool(name="ptbuf", bufs=2) as ptp,
            tc.tile_pool(name="rbuf", bufs=2) as rp,
            tc.tile_pool(name="dens", bufs=2) as densp,
            tc.tile_pool(name="ytbuf", bufs=8) as ytp,
            tc.tile_pool(name="outbuf", bufs=2) as outp,
        ):
            # ---- constants that need no DMA (feed the PE warmup) ----
            ident = constp.tile([128, 128], f32, name="ident")
            make_identity(nc, ident[:, :])
            ident_bf = constp.tile([128, 128], bf16, name="ident_bf")
            nc.vector.tensor_copy(ident_bf[:, :], ident[:, :])
            wrhs = constp.tile([128, BAND], bf16, name="wrhs")
            nc.vector.memset(wrhs[:, :], 0.0)

            # ---- DMAs in consumption order ----
            onesk_t = constp.tile([128, 128], bf16, name="onesk_t")
            nc.sync.dma_start(out=onesk_t[:, :], in_=onesk)
            bq_t = constp.tile([128, GROUP], f32, name="bq_t")
            nc.sync.dma_start(out=bq_t[:, :], in_=bq)

            # interleave wk quarters with x band-0 quarters so the first
            # K-projection matmuls start as soon as the first pair lands
            wk_t = wtsp.tile([128, NCH, 128], bf16, name="wk_t")
            b0q = []
            for c4 in range(4):
                nc.sync.dma_start(
                    out=wk_t[:, 4 * c4:4 * c4 + 4, :], in_=wk[:, 4 * c4:4 * c4 + 4, :]
                )
                t = xbp.tile([128, 4, BAND], bf16, name=f"b0q{c4}", tag="band0q",
                             bufs=4)
                nc.sync.dma_start(out=t[:, :, :], in_=xT[0, :, 4 * c4:4 * c4 + 4, :])
                b0q.append(t)

            wv_t = wtsp.tile([128, NCH, 128], bf16, name="wv_t")
            nc.sync.dma_start(out=wv_t[:, :, :], in_=wv)
            wq_t = []
            for g in range(GROUP):
                t = wtsp.tile([128, NCH, 128], bf16, name=f"wq_t{g}", tag=f"wq{g}")
                nc.sync.dma_start(out=t[:, :, :], in_=wq[g])
                wq_t.append(t)

            bands = [None] * NBAND
            for bd in range(1, NBAND):
                bands[bd] = xbp.tile(
                    [128, NCH, BAND], bf16, name=f"band{bd}", tag=f"band{bd}"
                )
                nc.sync.dma_start(out=bands[bd][:, :, :], in_=xT[bd])

            wo_t = []
            for g in range(GROUP):
                t = wtsp.tile([128, HIDDEN], bf16, name=f"wo_t{g}", tag=f"wo{g}")
                nc.sync.dma_start(out=t[:, :], in_=wo[g])
                wo_t.append(t)

            # ---- persistent activations ----
            qt_t = []
            for g in range(GROUP):
                t = qkvp.tile([128, S], bf16, name=f"qt{g}", tag=f"qt{g}")
                qt_t.append(t)
            kt_t = qkvp.tile([128, S], bf16, name="kt_t")
            v_t = qkvp.tile([128, S], bf16, name="v_t")
            vt_b = qkvp.tile([128, S], bf16, name="vt_b")

            def bch(bd, c):
                if bd == 0:
                    return b0q[c // 4][:, c % 4, :]
                return bands[bd][:, c, :]

            # ====== phase 1: K/V all bands + Q band 0 (Q bands 1-3 are
            # deferred into the attention passes as PE filler) ======
            with tc.tile_pool(name="psA", bufs=1, space="PSUM") as psA:
                # PE warmup: no data dependencies; fills the DMA startup
                # hole and gets HAM past the cold window
                pwarm = psA.tile([128, BAND], f32, name="pwarm", tag="warm")
                for _ in range(NWARM):
                    nc.tensor.matmul(
                        out=pwarm[:, :], lhsT=ident_bf[:, :], rhs=wrhs[:, :],
                        start=True, stop=True,
                    )

                for bd in range(NBAND):
                    bsl = slice(bd * BAND, (bd + 1) * BAND)

                    # K^T accumulation
                    pk = psA.tile([128, BAND], f32, name="pk", tag="pacc", bufs=3)
                    for c in range(NCH):
                        nc.tensor.matmul(
                            out=pk[:, :],
                            lhsT=wk_t[:, c, :],
                            rhs=bch(bd, c),
                            start=(c == 0), stop=(c == NCH - 1),
                        )
                    nc.scalar.activation(kt_t[:, bsl], pk[:, :], COPY)

                    # V^T accumulation (bf16, transposed to V per 128-block)
                    pv = psA.tile([128, BAND], f32, name="pv", tag="pacc", bufs=3)
                    for c in range(NCH):
                        nc.tensor.matmul(
                            out=pv[:, :],
                            lhsT=wv_t[:, c, :],
                            rhs=bch(bd, c),
                            start=(c == 0), stop=(c == NCH - 1),
                        )
                    nc.scalar.activation(vt_b[:, bsl], pv[:, :], COPY)

                    # Q^T per local head, band 0 only
                    if bd == 0:
                        for g in range(GROUP):
                            pq = psA.tile([128, BAND], f32, name="pq", tag="pacc",
                                          bufs=3)
                            for c in range(NCH):
                                nc.tensor.matmul(
                                    out=pq[:, :],
                                    lhsT=wq_t[g][:, c, :],
                                    rhs=bch(0, c),
                                    start=(c == 0), stop=(c == NCH - 1),
                                )
                            nc.scalar.activation(
                                qt_t[g][:, bsl], pq[:, :], IDENT,
                                bias=bq_t[:, g:g + 1],
                            )

                    # transpose V^T band -> V (bf16 transpose, 1 cyc/row)
                    for t in range(BAND // 128):
                        sk = bd * (BAND // 128) + t
                        pt = psA.tile([128, 128], bf16, name="ptr", tag="ptr", bufs=2)
                        nc.tensor.transpose(
                            pt[:, :], vt_b[:, sk * 128:(sk + 1) * 128],
                            ident_bf[:, :]
                        )
                        nc.scalar.activation(
                            v_t[:, sk * 128:(sk + 1) * 128], pt[:, :], COPY
                        )

            # ====== phase 2: attention w/ interleaved filler + outproj ======
            # psB tags: sc x2 + py(yacc) x2 + pden x1 + aux x3 = 8 banks
            with tc.tile_pool(name="psB", bufs=1, space="PSUM") as psB:
                yt_all = {}
                pending = [None]  # deferred den+normalize closure

                def flush_pending():
                    if pending[0] is not None:
                        pending[0]()
                        pending[0] = None

                def aux_tile(name):
                    return psB.tile([128, BAND], f32, name=name, tag="aux", bufs=3)

                def attn_group(qt, g, fillers=(), tail_fillers=()):
                    """One head-group of attention over qtile qt.

                    fillers: callbacks fill(sk) issuing exactly one PE matmul
                    each, called once per sk in 0..NSK-1 (16 calls); used for
                    deferred Q-band projections or interleaved out-projection
                    chunks. tail_fillers run after the trailing AVs.
                    """
                    qsl = slice(qt * QTILE, (qt + 1) * QTILE)
                    py = psB.tile([128, QTILE], f32, name="py", tag="yacc", bufs=2)
                    pt_all = ptp.tile([128, NSK * QTILE], bf16, name="pt", tag="pt")

                    def pt(a):
                        return pt_all[:, a * QTILE:(a + 1) * QTILE]

                    # DVE accumulation chain in bf16 for the softmax denominator
                    R = rp.tile([128, QTILE], bf16, name="racc", tag="racc")

                    for sk in range(NSK):
                        ps = psB.tile([128, QTILE], f32, name="ps", tag="sc", bufs=2)
                        nc.tensor.matmul(
                            out=ps[:, :],
                            lhsT=kt_t[:, sk * 128:(sk + 1) * 128],
                            rhs=qt_t[g][:, qsl],
                            start=True, stop=True,
                        )
                        nc.scalar.activation(pt(sk), ps[:, :], EXP, scale=SCALE)
                        if sk == 1:
                            nc.vector.tensor_add(R[:, :], pt(0), pt(1))
                        elif sk >= 2:
                            nc.vector.tensor_add(R[:, :], R[:, :], pt(sk))
                        if sk >= 2:
                            # AV trails scores by two sk so exp latency and
                            # jitter stay hidden
                            psk = sk - 2
                            nc.tensor.matmul(
                                out=py[:, :],
                                lhsT=v_t[:, psk * 128:(psk + 1) * 128],
                                rhs=pt(psk),
                                start=(psk == 0), stop=False,
                            )
                        for fill in fillers:
                            fill(sk)
                        if sk == 2:
                            flush_pending()
                    for psk in (NSK - 2, NSK - 1):
                        nc.tensor.matmul(
                            out=py[:, :],
                            lhsT=v_t[:, psk * 128:(psk + 1) * 128],
                            rhs=pt(psk),
                            start=False, stop=(psk == NSK - 1),
                        )
                    for tf in tail_fillers:
                        tf()

                    def normalize(qt=qt, g=g, py=py, R=R):
                        # all-ones stationary: out[i, q] = sum_p R[p, q] —
                        # partition-reduces AND replicates den on all 128
                        # partitions in one matmul
                        pden = psB.tile([128, QTILE], f32, name="pden", tag="pden")
                        nc.tensor.matmul(
                            out=pden[:, :],
                            lhsT=onesk_t[:, :],
                            rhs=R[:, :],
                            start=True, stop=True,
                        )
                        rb = densp.tile([128, QTILE], f32, name="rb", tag="rb")
                        nc.vector.reciprocal_approx_fast(rb[:, :], pden[:, :])
                        yt = ytp.tile([128, QTILE], bf16, name="yt", tag="yt")
                        nc.vector.tensor_mul(yt[:, :], py[:, :], rb[:, :])
                        yt_all[(qt, g)] = yt

                    pending[0] = normalize

                def make_qproj_filler(bd, g):
                    """Deferred Q-projection of (band bd, head g): one
                    contraction-chunk matmul per sk into an aux psum bank,
                    evacuated with the bias add at the end of the pass."""
                    qacc = aux_tile(f"qacc{bd}")
                    bsl = slice(bd * BAND, (bd + 1) * BAND)

                    def fill(sk):
                        nc.tensor.matmul(
                            out=qacc[:, :],
                            lhsT=wq_t[g][:, sk, :],
                            rhs=bands[bd][:, sk, :],
                            start=(sk == 0), stop=(sk == NCH - 1),
                        )

                    def evac():
                        nc.scalar.activation(
                            qt_t[g][:, bsl], qacc[:, :], IDENT,
                            bias=bq_t[:, g:g + 1],
                        )

                    return fill, evac

                def make_outproj_filler(proj_qt, i):
                    """Out-projection block (proj_qt, i): j-chunk sk//4 is
                    accumulated over the 4 heads during sk%4, then copied
                    out (DVE) and DMA'd per 512-column chunk."""
                    outs = outp.tile([128, HIDDEN], bf16, name="outs", tag="outs")
                    state = {}

                    def fill(sk):
                        j, gp = sk // 4, sk % 4
                        if gp == 0:
                            state["po"] = aux_tile("po")
                        po = state["po"]
                        nc.tensor.matmul(
                            out=po[:, :],
                            lhsT=yt_all[(proj_qt, gp)][:, i * 128:(i + 1) * 128],
                            rhs=wo_t[gp][:, j * 512:(j + 1) * 512],
                            start=(gp == 0), stop=(gp == GROUP - 1),
                        )
                        if gp == GROUP - 1:
                            csl = slice(j * 512, (j + 1) * 512)
                            nc.vector.tensor_copy(outs[:, csl], po[:, :])
                            r0 = proj_qt * QTILE + i * 128
                            nc.sync.dma_start(
                                out=out[r0:r0 + 128, j * 512:(j + 1) * 512],
                                in_=outs[:, csl],
                            )

                    return fill

                # qt0: + Q-projections of bands 1 and 2 (2 filler MMs/sk)
                for g in range(GROUP):
                    f1, e1 = make_qproj_filler(1, g)
                    f2, e2 = make_qproj_filler(2, g)
                    attn_group(0, g, fillers=(f1, f2), tail_fillers=(e1, e2))
                # qt1: + Q-projection of band 3
                for g in range(GROUP):
                    f3, e3 = make_qproj_filler(3, g)
                    attn_group(1, g, fillers=(f3,), tail_fillers=(e3,))
                # qt2/qt3: + out-projection of qt0/qt1 (i = g)
                for g in range(GROUP):
                    attn_group(2, g, fillers=(make_outproj_filler(0, g),))
                for g in range(GROUP):
                    attn_group(3, g, fillers=(make_outproj_filler(1, g),))
                flush_pending()

                # tail: out-projection of qt2 and qt3, pure PE
                for proj_qt in (2, 3):
                    for i in range(QTILE // 128):
                        outs = outp.tile(
                            [128, HIDDEN], bf16, name="outs", tag="outs"
                        )
                        for j in range(4):
                            po = aux_tile("po")
                            for gp in range(GROUP):
                                nc.tensor.matmul(
                                    out=po[:, :],
                                    lhsT=yt_all[(proj_qt, gp)][
                                        :, i * 128:(i + 1) * 128],
                                    rhs=wo_t[gp][:, j * 512:(j + 1) * 512],
                                    start=(gp == 0), stop=(gp == GROUP - 1),
                                )
                            csl = slice(j * 512, (j + 1) * 512)
                            nc.vector.tensor_copy(outs[:, csl], po[:, :])
                            r0 = proj_qt * QTILE + i * 128
                            nc.sync.dma_start(
                                out=out[r0:r0 + 128, j * 512:(j + 1) * 512],
                                in_=outs[:, csl],
                            )

    nc.finalize()
    return nc


def _get_nc():
    if "nc" not in _CACHE:
        _CACHE["nc"] = _build()
    return _CACHE["nc"]


def kernel(x, Wq, bq, Wk, bk, Wv, bv, Wo, bo):
    global LAST_RESULTS
    from concourse.bass_utils import run_bass_kernel_spmd

    bf = ml_dtypes.bfloat16
    x = np.asarray(x, np.float32)
    Wq = np.asarray(Wq, np.float32)
    Wk = np.asarray(Wk, np.float32)
    Wv = np.asarray(Wv, np.float32)
    Wo = np.asarray(Wo, np.float32)
    bq = np.asarray(bq, np.float32)
    bv = np.asarray(bv, np.float32)
    bo = np.asarray(bo, np.float32)

    nc = _get_nc()

    onesk_np = np.ones((128, 128), bf)

    in_maps = []
    for c in range(8):
        b, h = divmod(c, NKV)
        xTb = x[b].T  # [HIDDEN, S]
        xTh = np.ascontiguousarray(
            xTb.reshape(NCH, 128, NBAND, BAND).transpose(2, 1, 0, 3)
        ).astype(bf)
        # wq[g]: [128, NCH, 128] per local head
        wqh = np.ascontiguousarray(
            Wq[:, h * 512:(h + 1) * 512]
            .reshape(NCH, 128, GROUP, 128).transpose(2, 1, 0, 3)
        ).astype(bf)
        wkh = np.ascontiguousarray(
            Wk[:, h * 128:(h + 1) * 128].reshape(NCH, 128, 128).transpose(1, 0, 2)
        ).astype(bf)
        wvh = np.ascontiguousarray(
            Wv[:, h * 128:(h + 1) * 128].reshape(NCH, 128, 128).transpose(1, 0, 2)
        ).astype(bf)
        woh = np.ascontiguousarray(
            Wo[h * 512:(h + 1) * 512, :].reshape(GROUP, 128, HIDDEN)
        ).astype(bf)
        bqh = np.ascontiguousarray(
            bq[h * 512:(h + 1) * 512].reshape(GROUP, 128).T
        )
        in_maps.append({
            "xT": xTh, "wq": wqh, "wk": wkh, "wv": wvh, "wo": woh,
            "bq": bqh, "onesk": onesk_np,
        })

    res = run_bass_kernel_spmd(
        nc, in_maps, list(range(8)), trace=TRACE, tmpdir=TMPDIR
    )
    LAST_RESULTS = res

    # host-side constant bias: (bv repeated per head group) @ Wo + bo
    bv_rep = np.broadcast_to(
        bv.reshape(NKV, 1, D), (NKV, GROUP, D)
    ).reshape(HIDDEN)
    bias_row = bv_rep @ Wo + bo  # [HIDDEN]

    out = np.empty((B, S, HIDDEN), np.float32)
    for b in range(B):
        acc = res.results[b * NKV + 0]["out"].astype(np.float32)
        for h in range(1, NKV):
            acc = acc + res.results[b * NKV + h]["out"].astype(np.float32)
        out[b] = acc + bias_row
    return out


# revision 3
# speedup vs baseline: 1.1995x; 1.0020x over previous
"""GQA attention kernel for 8 Trainium2 NeuronCores — v3.

Sharding: core c = 4*b + h handles batch b (of 2) and kv-head h (of 4),
i.e. one kv head + its 4 grouped query heads. Each core computes its head
group's partial contribution to the output projection; the host sums the
4 partials per batch. No collectives.

v3 changes vs v2 (404us traced):
- The PE is the global bottleneck (~1200 N=512 matmuls x 259ns = ~311us
  at the observed 2.0 GHz sustained clock); every other engine has slack.
  So the entire schedule is rebuilt around "PE never idles":
- Q projections for S-bands 1-3 are DEFERRED out of the projection phase
  and interleaved as per-sk filler matmuls inside the attention passes
  (qt0 gets bands 1+2 = 2 filler MMs/sk, qt1 gets band 3). This kills
  v2's qt0 "pair mode", whose 1-buf psum pair serialized everything
  behind each EXP (PE duty ~30%, HAM-cold matmuls at 2x duration,
  ~42us lost).
- Out-projection of qt0/qt1 interleaves into qt2/qt3 (one j-chunk at a
  time through a 3-buf psum rotation); outproj of qt2/qt3 runs as a
  dense pure-PE tail. Total PE work is unchanged; only placed so no
  attention pass is EXP-bound.
- 12 warm-up matmuls on an identity tile (no DMA dependency) fill the
  ~9us DMA startup hole and get the PE past the HAM cold window before
  real data lands; wk is DMA'd in 4 chunk-quarters so the first real
  matmul starts on the first quarter.
- Output partials in bf16 (halves output DMA; host sums in f32);
  V^T transposed in bf16 (1 cyc/row instead of 2).
- Single attention-phase PSUM pool: sc x2 + py x2 + pden x1 + aux x3 = 8
  banks; aux serves qt0/qt1 Q-proj accumulators and qt2/qt3+tail
  outproj chunks through rotation.

Device math per core (S=2048, H=2048, d=128):
  QT_g = (x @ Wq_g + bq_g)^T          [d, S]   g=0..3   (bf16)
  KT   = (x @ Wk_h)^T                 [d, S]            (bk cancels in softmax)
  V    = x @ Wv_h                     [S, d]   (computed as V^T then PE-transposed)
  S^T  = KT^T-blocks @ QT             [Sk, Sq]
  P^T  = exp(SCALE * S^T)             (bf16, no max-subtraction: |s| <~ 6)
  den  = ones.T @ (DVE-sum of P^T tiles)
  y^T  = V^T-blocks.T @ P^T (PSUM accum);  yT := y^T * (1/den)
  out += yT_g^T @ Wo_g                [S, H]  (partial over this head group)
Host: out[b] = sum_h partial + (bv_rep @ Wo + bo).
"""

import numpy as np
import ml_dtypes

B = 2
S = 2048
HIDDEN = 2048
NKV = 4
GROUP = 4
D = 128
SCALE = D ** -0.5

BAND = 512            # S-columns per projection band
NBAND = S // BAND     # 4
NCH = HIDDEN // 128   # 16 contraction chunks
QTILE = 512           # queries per attention tile
NQT = S // QTILE      # 4
NSK = S // 128        # 16 key tiles
NWARM = 12            # PE warmup matmuls during DMA startup

_CACHE = {}
LAST_RESULTS = None
TRACE = False
TMPDIR = None


def _build():
    import concourse.bass as bass
    import concourse.bacc as bacc
    import concourse.mybir as mybir
    import concourse.tile as tile
    from concourse.masks import make_identity

    f32 = mybir.dt.float32
    bf16 = mybir.dt.bfloat16
    EXP = mybir.ActivationFunctionType.Exp
    IDENT = mybir.ActivationFunctionType.Identity
    COPY = mybir.ActivationFunctionType.Copy

    nc = bacc.Bacc(trn_type="TRN2", target_bir_lowering=False, debug=False)

    xT = nc.dram_tensor("xT", [NBAND, 128, NCH, BAND], bf16, kind="ExternalInput").ap()
    wq = nc.dram_tensor("wq", [GROUP, 128, NCH, 128], bf16, kind="ExternalInput").ap()
    wk = nc.dram_tensor("wk", [128, NCH, 128], bf16, kind="ExternalInput").ap()
    wv = nc.dram_tensor("wv", [128, NCH, 128], bf16, kind="ExternalInput").ap()
    wo = nc.dram_tensor("wo", [GROUP, 128, HIDDEN], bf16, kind="ExternalInput").ap()
    bq = nc.dram_tensor("bq", [128, GROUP], f32, kind="ExternalInput").ap()
    onesk = nc.dram_tensor("onesk", [128, 128], bf16, kind="ExternalInput").ap()
    out = nc.dram_tensor("out", [S, HIDDEN], bf16, kind="ExternalOutput").ap()

    with tile.TileContext(nc) as tc:
        with (
            tc.tile_pool(name="const", bufs=1) as constp,
            tc.tile_pool(name="wts", bufs=1) as wtsp,
            tc.tile_pool(name="xb", bufs=1) as xbp,
            tc.tile_pool(name="qkv", bufs=1) as qkvp,
            tc.tile_pool(name="ptbuf", bufs=2) as ptp,
            tc.tile_pool(name="rbuf", bufs=2) as rp,
            tc.tile_pool(name="dens", bufs=2) as densp,
            tc.tile_pool(name="ytbuf", bufs=8) as ytp,
            tc.tile_pool(name="outbuf", bufs=2) as outp,
        ):
            # ---- constants that need no DMA (feed the PE warmup) ----
            ident = constp.tile([128, 128], f32, name="ident")
            make_identity(nc, ident[:, :])
            ident_bf = constp.tile([128, 128], bf16, name="ident_bf")
            nc.vector.tensor_copy(ident_bf[:, :], ident[:, :])
            wrhs = constp.tile([128, BAND], bf16, name="wrhs")
            nc.vector.memset(wrhs[:, :], 0.0)

            # ---- DMAs in consumption order ----
            onesk_t = constp.tile([128, 128], bf16, name="onesk_t")
            nc.sync.dma_start(out=onesk_t[:, :], in_=onesk)
            bq_t = constp.tile([128, GROUP], f32, name="bq_t")
            nc.sync.dma_start(out=bq_t[:, :], in_=bq)

            # interleave wk quarters with x band-0 quarters so the first
            # K-projection matmuls start as soon as the first pair lands
            wk_t = wtsp.tile([128, NCH, 128], bf16, name="wk_t")
            b0q = []
            for c4 in range(4):
                nc.sync.dma_start(
                    out=wk_t[:, 4 * c4:4 * c4 + 4, :], in_=wk[:, 4 * c4:4 * c4 + 4, :]
                )
                t = xbp.tile([128, 4, BAND], bf16, name=f"b0q{c4}", tag="band0q",
                             bufs=4)
                nc.sync.dma_start(out=t[:, :, :], in_=xT[0, :, 4 * c4:4 * c4 + 4, :])
                b0q.append(t)

            wv_t = wtsp.tile([128, NCH, 128], bf16, name="wv_t")
            nc.sync.dma_start(out=wv_t[:, :, :], in_=wv)
            wq_t = []
            for g in range(GROUP):
                t = wtsp.tile([128, NCH, 128], bf16, name=f"wq_t{g}", tag=f"wq{g}")
                nc.sync.dma_start(out=t[:, :, :], in_=wq[g])
                wq_t.append(t)

            bands = [None] * NBAND
            for bd in range(1, NBAND):
                bands[bd] = xbp.tile(
                    [128, NCH, BAND], bf16, name=f"band{bd}", tag=f"band{bd}"
                )
                nc.sync.dma_start(out=bands[bd][:, :, :], in_=xT[bd])

            wo_t = []
            for g in range(GROUP):
                t = wtsp.tile([128, HIDDEN], bf16, name=f"wo_t{g}", tag=f"wo{g}")
                nc.sync.dma_start(out=t[:, :], in_=wo[g])
                wo_t.append(t)

            # ---- persistent activations ----
            qt_t = []
            for g in range(GROUP):
                t = qkvp.tile([128, S], bf16, name=f"qt{g}", tag=f"qt{g}")
                qt_t.append(t)
            kt_t = qkvp.tile([128, S], bf16, name="kt_t")
            v_t = qkvp.tile([128, S], bf16, name="v_t")
            vt_b = qkvp.tile([128, S], bf16, name="vt_b")

            def bch(bd, c):
                if bd == 0:
                    return b0q[c // 4][:, c % 4, :]
                return bands[bd][:, c, :]

            # ====== phase 1: K/V all bands + Q band 0 (Q bands 1-3 are
            # deferred into the attention passes as PE filler) ======
            with tc.tile_pool(name="psA", bufs=1, space="PSUM") as psA:
                # PE warmup: no data dependencies; fills the DMA startup
                # hole and gets HAM past the cold window
                pwarm = psA.tile([128, BAND], f32, name="pwarm", tag="warm")
                for _ in range(NWARM):
                    nc.tensor.matmul(
                        out=pwarm[:, :], lhsT=ident_bf[:, :], rhs=wrhs[:, :],
                        start=True, stop=True,
                    )

                for bd in range(NBAND):
                    bsl = slice(bd * BAND, (bd + 1) * BAND)

                    # K^T accumulation
                    pk = psA.tile([128, BAND], f32, name="pk", tag="pacc", bufs=3)
                    for c in range(NCH):
                        nc.tensor.matmul(
                            out=pk[:, :],
                            lhsT=wk_t[:, c, :],
                            rhs=bch(bd, c),
                            start=(c == 0), stop=(c == NCH - 1),
                        )
                    nc.scalar.activation(kt_t[:, bsl], pk[:, :], COPY)

                    # V^T accumulation (bf16, transposed to V per 128-block)
                    pv = psA.tile([128, BAND], f32, name="pv", tag="pacc", bufs=3)
                    for c in range(NCH):
                        nc.tensor.matmul(
                            out=pv[:, :],
                            lhsT=wv_t[:, c, :],
                            rhs=bch(bd, c),
                            start=(c == 0), stop=(c == NCH - 1),
                        )
                    nc.scalar.activation(vt_b[:, bsl], pv[:, :], COPY)

                    # Q^T per local head, band 0 only
                    if bd == 0:
                        for g in range(GROUP):
                            pq = psA.tile([128, BAND], f32, name="pq", tag="pacc",
                                          bufs=3)
                            for c in range(NCH):
                                nc.tensor.matmul(
                                    out=pq[:, :],
                                    lhsT=wq_t[g][:, c, :],
                                    rhs=bch(0, c),
                                    start=(c == 0), stop=(c == NCH - 1),
                                )
                            nc.scalar.activation(
                                qt_t[g][:, bsl], pq[:, :], IDENT,
                                bias=bq_t[:, g:g + 1],
                            )

                    # transpose V^T band -> V (bf16 transpose, 1 cyc/row)
                    for t in range(BAND // 128):
                        sk = bd * (BAND // 128) + t
                        pt = psA.tile([128, 128], bf16, name="ptr", tag="ptr", bufs=2)
                        nc.tensor.transpose(
                            pt[:, :], vt_b[:, sk * 128:(sk + 1) * 128],
                            ident_bf[:, :]
                        )
                        nc.scalar.activation(
                            v_t[:, sk * 128:(sk + 1) * 128], pt[:, :], COPY
                        )

            # ====== phase 2: attention w/ interleaved filler + outproj ======
            # psB tags: sc x3 + py(yacc) x2 + aux x3 = 8 banks; pden shares
            # the aux rotation (it is live only ~1us inside the deferred
            # flush, and the per-pass aux call counts keep reuse distances
            # of >=9 sk for every rotation slot)
            with tc.tile_pool(name="psB", bufs=1, space="PSUM") as psB:
                yt_all = {}
                pending = [None]  # deferred den+normalize closure

                def flush_pending():
                    if pending[0] is not None:
                        pending[0]()
                        pending[0] = None

                def aux_tile(name):
                    return psB.tile([128, BAND], f32, name=name, tag="aux", bufs=3)

                def attn_group(qt, g, fillers=(), tail_fillers=()):
                    """One head-group of attention over qtile qt.

                    fillers: callbacks fill(sk) issuing exactly one PE matmul
                    each, called once per sk in 0..NSK-1 (16 calls); used for
                    deferred Q-band projections or interleaved out-projection
                    chunks. tail_fillers run after the trailing AVs.
                    """
                    qsl = slice(qt * QTILE, (qt + 1) * QTILE)
                    py = psB.tile([128, QTILE], f32, name="py", tag="yacc", bufs=2)
                    pt_all = ptp.tile([128, NSK * QTILE], bf16, name="pt", tag="pt")

                    def pt(a):
                        return pt_all[:, a * QTILE:(a + 1) * QTILE]

                    # DVE accumulation chain in bf16 for the softmax denominator
                    R = rp.tile([128, QTILE], bf16, name="racc", tag="racc")

                    for sk in range(NSK):
                        ps = psB.tile([128, QTILE], f32, name="ps", tag="sc", bufs=2)
                        nc.tensor.matmul(
                            out=ps[:, :],
                            lhsT=kt_t[:, sk * 128:(sk + 1) * 128],
                            rhs=qt_t[g][:, qsl],
                            start=True, stop=True,
                        )
                        nc.scalar.activation(pt(sk), ps[:, :], EXP, scale=SCALE)
                        if sk == 1:
                            nc.vector.tensor_add(R[:, :], pt(0), pt(1))
                        elif sk >= 2:
                            nc.vector.tensor_add(R[:, :], R[:, :], pt(sk))
                        if sk >= 2:
                            # AV trails scores by two sk so exp latency and
                            # jitter stay hidden
                            psk = sk - 2
                            nc.tensor.matmul(
                                out=py[:, :],
                                lhsT=v_t[:, psk * 128:(psk + 1) * 128],
                                rhs=pt(psk),
                                start=(psk == 0), stop=False,
                            )
                        for fill in fillers:
                            fill(sk)
                        if sk == 2:
                            flush_pending()
                    for psk in (NSK - 2, NSK - 1):
                        nc.tensor.matmul(
                            out=py[:, :],
                            lhsT=v_t[:, psk * 128:(psk + 1) * 128],
                            rhs=pt(psk),
                            start=False, stop=(psk == NSK - 1),
                        )
                    for tf in tail_fillers:
                        tf()

                    def normalize(qt=qt, g=g, py=py, R=R):
                        # all-ones stationary: out[i, q] = sum_p R[p, q] —
                        # partition-reduces AND replicates den on all 128
                        # partitions in one matmul
                        pden = psB.tile([128, QTILE], f32, name="pden", tag="pden")
                        nc.tensor.matmul(
                            out=pden[:, :],
                            lhsT=onesk_t[:, :],
                            rhs=R[:, :],
                            start=True, stop=True,
                        )
                        rb = densp.tile([128, QTILE], f32, name="rb", tag="rb")
                        nc.vector.reciprocal_approx_fast(rb[:, :], pden[:, :])
                        yt = ytp.tile([128, QTILE], bf16, name="yt", tag="yt")
                        nc.vector.tensor_mul(yt[:, :], py[:, :], rb[:, :])
                        yt_all[(qt, g)] = yt

                    pending[0] = normalize

                def make_qproj_filler(bd, g):
                    """Deferred Q-projection of (band bd, head g): one
                    contraction-chunk matmul per sk into an aux psum bank,
                    evacuated with the bias add at the end of the pass."""
                    qacc = aux_tile(f"qacc{bd}")
                    bsl = slice(bd * BAND, (bd + 1) * BAND)

                    def fill(sk):
                        nc.tensor.matmul(
                            out=qacc[:, :],
                            lhsT=wq_t[g][:, sk, :],
                            rhs=bands[bd][:, sk, :],
                            start=(sk == 0), stop=(sk == NCH - 1),
                        )

                    def evac():
                        nc.scalar.activation(
                            qt_t[g][:, bsl], qacc[:, :], IDENT,
                            bias=bq_t[:, g:g + 1],
                        )

                    return fill, evac

                def make_outproj_filler(proj_qt, i):
                    """Out-projection block (proj_qt, i): j-chunk sk//4 is
                    accumulated over the 4 heads during sk%4, then copied
                    out (DVE) and DMA'd per 512-column chunk."""
                    outs = outp.tile([128, HIDDEN], bf16, name="outs", tag="outs")
                    state = {}

                    def fill(sk):
                        j, gp = sk // 4, sk % 4
                        if gp == 0:
                            state["po"] = aux_tile("po")
                        po = state["po"]
                        nc.tensor.matmul(
                            out=po[:, :],
                            lhsT=yt_all[(proj_qt, gp)][:, i * 128:(i + 1) * 128],
                            rhs=wo_t[gp][:, j * 512:(j + 1) * 512],
                            start=(gp == 0), stop=(gp == GROUP - 1),
                        )
                        if gp == GROUP - 1:
                            csl = slice(j * 512, (j + 1) * 512)
                            nc.vector.tensor_copy(outs[:, csl], po[:, :])
                            r0 = proj_qt * QTILE + i * 128
                            nc.sync.dma_start(
                                out=out[r0:r0 + 128, j * 512:(j + 1) * 512],
                                in_=outs[:, csl],
                            )

                    return fill

                # qt0: + Q-projections of bands 1 and 2 (2 filler MMs/sk)
                for g in range(GROUP):
                    f1, e1 = make_qproj_filler(1, g)
                    f2, e2 = make_qproj_filler(2, g)
                    attn_group(0, g, fillers=(f1, f2), tail_fillers=(e1, e2))
                # qt1: + Q-projection of band 3
                for g in range(GROUP):
                    f3, e3 = make_qproj_filler(3, g)
                    attn_group(1, g, fillers=(f3,), tail_fillers=(e3,))
                # qt2/qt3: + out-projection of qt0/qt1 (i = g)
                for g in range(GROUP):
                    attn_group(2, g, fillers=(make_outproj_filler(0, g),))
                for g in range(GROUP):
                    attn_group(3, g, fillers=(make_outproj_filler(1, g),))
                flush_pending()

                # tail: out-projection of qt2 and qt3, pure PE
                for proj_qt in (2, 3):
                    for i in range(QTILE // 128):
                        outs = outp.tile(
                            [128, HIDDEN], bf16, name="outs", tag="outs"
                        )
                        for j in range(4):
                            po = aux_tile("po")
                            for gp in range(GROUP):
                                nc.tensor.matmul(
                                    out=po[:, :],
                                    lhsT=yt_all[(proj_qt, gp)][
                                        :, i * 128:(i + 1) * 128],
                                    rhs=wo_t[gp][:, j * 512:(j + 1) * 512],
                                    start=(gp == 0), stop=(gp == GROUP - 1),
                                )
                            csl = slice(j * 512, (j + 1) * 512)
                            nc.vector.tensor_copy(outs[:, csl], po[:, :])
                            r0 = proj_qt * QTILE + i * 128
                            nc.sync.dma_start(
                                out=out[r0:r0 + 128, j * 512:(j + 1) * 512],
                                in_=outs[:, csl],
                            )

    nc.finalize()
    return nc


def _get_nc():
    if "nc" not in _CACHE:
        _CACHE["nc"] = _build()
    return _CACHE["nc"]


def kernel(x, Wq, bq, Wk, bk, Wv, bv, Wo, bo):
    global LAST_RESULTS
    from concourse.bass_utils import run_bass_kernel_spmd

    bf = ml_dtypes.bfloat16
    x = np.asarray(x, np.float32)
    Wq = np.asarray(Wq, np.float32)
    Wk = np.asarray(Wk, np.float32)
    Wv = np.asarray(Wv, np.float32)
    Wo = np.asarray(Wo, np.float32)
    bq = np.asarray(bq, np.float32)
    bv = np.asarray(bv, np.float32)
    bo = np.asarray(bo, np.float32)

    nc = _get_nc()

    onesk_np = np.ones((128, 128), bf)

    in_maps = []
    for c in range(8):
        b, h = divmod(c, NKV)
        xTb = x[b].T  # [HIDDEN, S]
        xTh = np.ascontiguousarray(
            xTb.reshape(NCH, 128, NBAND, BAND).transpose(2, 1, 0, 3)
        ).astype(bf)
        # wq[g]: [128, NCH, 128] per local head
        wqh = np.ascontiguousarray(
            Wq[:, h * 512:(h + 1) * 512]
            .reshape(NCH, 128, GROUP, 128).transpose(2, 1, 0, 3)
        ).astype(bf)
        wkh = np.ascontiguousarray(
            Wk[:, h * 128:(h + 1) * 128].reshape(NCH, 128, 128).transpose(1, 0, 2)
        ).astype(bf)
        wvh = np.ascontiguousarray(
            Wv[:, h * 128:(h + 1) * 128].reshape(NCH, 128, 128).transpose(1, 0, 2)
        ).astype(bf)
        woh = np.ascontiguousarray(
            Wo[h * 512:(h + 1) * 512, :].reshape(GROUP, 128, HIDDEN)
        ).astype(bf)
        bqh = np.ascontiguousarray(
            bq[h * 512:(h + 1) * 512].reshape(GROUP, 128).T
        )
        in_maps.append({
            "xT": xTh, "wq": wqh, "wk": wkh, "wv": wvh, "wo": woh,
            "bq": bqh, "onesk": onesk_np,
        })

    res = run_bass_kernel_spmd(
        nc, in_maps, list(range(8)), trace=TRACE, tmpdir=TMPDIR
    )
    LAST_RESULTS = res

    # host-side constant bias: (bv repeated per head group) @ Wo + bo
    bv_rep = np.broadcast_to(
        bv.reshape(NKV, 1, D), (NKV, GROUP, D)
    ).reshape(HIDDEN)
    bias_row = bv_rep @ Wo + bo  # [HIDDEN]

    out = np.empty((B, S, HIDDEN), np.float32)
    for b in range(B):
        acc = res.results[b * NKV + 0]["out"].astype(np.float32)
        for h in range(1, NKV):
            acc = acc + res.results[b * NKV + h]["out"].astype(np.float32)
        out[b] = acc + bias_row
    return out


# revision 5
# speedup vs baseline: 1.3841x; 1.1539x over previous
"""GQA attention kernel for 8 Trainium2 NeuronCores — v3.

Sharding: core c = 4*b + h handles batch b (of 2) and kv-head h (of 4),
i.e. one kv head + its 4 grouped query heads. Each core computes its head
group's partial contribution to the output projection; the host sums the
4 partials per batch. No collectives.

v3 changes vs v2 (404us traced):
- The PE is the global bottleneck (~1200 N=512 matmuls x 259ns = ~311us
  at the observed 2.0 GHz sustained clock); every other engine has slack.
  So the entire schedule is rebuilt around "PE never idles":
- Q projections for S-bands 1-3 are DEFERRED out of the projection phase
  and interleaved as per-sk filler matmuls inside the attention passes
  (qt0 gets bands 1+2 = 2 filler MMs/sk, qt1 gets band 3). This kills
  v2's qt0 "pair mode", whose 1-buf psum pair serialized everything
  behind each EXP (PE duty ~30%, HAM-cold matmuls at 2x duration,
  ~42us lost).
- Out-projection of qt0/qt1 interleaves into qt2/qt3 (one j-chunk at a
  time through a 3-buf psum rotation); outproj of qt2/qt3 runs as a
  dense pure-PE tail. Total PE work is unchanged; only placed so no
  attention pass is EXP-bound.
- 12 warm-up matmuls on an identity tile (no DMA dependency) fill the
  ~9us DMA startup hole and get the PE past the HAM cold window before
  real data lands; wk is DMA'd in 4 chunk-quarters so the first real
  matmul starts on the first quarter.
- Output partials in bf16 (halves output DMA; host sums in f32);
  V^T transposed in bf16 (1 cyc/row instead of 2).
- Single attention-phase PSUM pool: sc x2 + py x2 + pden x1 + aux x3 = 8
  banks; aux serves qt0/qt1 Q-proj accumulators and qt2/qt3+tail
  outproj chunks through rotation.

Device math per core (S=2048, H=2048, d=128):
  QT_g = (x @ Wq_g + bq_g)^T          [d, S]   g=0..3   (bf16)
  KT   = (x @ Wk_h)^T                 [d, S]            (bk cancels in softmax)
  V    = x @ Wv_h                     [S, d]   (computed as V^T then PE-transposed)
  S^T  = KT^T-blocks @ QT             [Sk, Sq]
  P^T  = exp(SCALE * S^T)             (bf16, no max-subtraction: |s| <~ 6)
  den  = ones.T @ (DVE-sum of P^T tiles)
  y^T  = V^T-blocks.T @ P^T (PSUM accum);  yT := y^T * (1/den)
  out += yT_g^T @ Wo_g                [S, H]  (partial over this head group)
Host: out[b] = sum_h partial + (bv_rep @ Wo + bo).
"""

import numpy as np
import ml_dtypes

B = 2
S = 2048
HIDDEN = 2048
NKV = 4
GROUP = 4
D = 128
SCALE = D ** -0.5

BAND = 512            # S-columns per projection band
NBAND = S // BAND     # 4
NCH = HIDDEN // 128   # 16 contraction chunks
QTILE = 512           # queries per attention tile
NQT = S // QTILE      # 4
NSK = S // 128        # 16 key tiles
NWARM = 12            # PE warmup matmuls during DMA startup

_CACHE = {}
LAST_RESULTS = None
TRACE = False
TMPDIR = None


def _build():
    import concourse.bass as bass
    import concourse.bacc as bacc
    import concourse.mybir as mybir
    import concourse.tile as tile
    from concourse.masks import make_identity

    f32 = mybir.dt.float32
    bf16 = mybir.dt.bfloat16
    EXP = mybir.ActivationFunctionType.Exp
    IDENT = mybir.ActivationFunctionType.Identity
    COPY = mybir.ActivationFunctionType.Copy

    nc = bacc.Bacc(trn_type="TRN2", target_bir_lowering=False, debug=False)

    xT = nc.dram_tensor("xT", [NBAND, 128, NCH, BAND], bf16, kind="ExternalInput").ap()
    wq = nc.dram_tensor("wq", [GROUP, 128, NCH, 128], bf16, kind="ExternalInput").ap()
    wk = nc.dram_tensor("wk", [128, NCH, 128], bf16, kind="ExternalInput").ap()
    wv = nc.dram_tensor("wv", [128, NCH, 128], bf16, kind="ExternalInput").ap()
    wo = nc.dram_tensor("wo", [GROUP, 128, HIDDEN], bf16, kind="ExternalInput").ap()
    bq = nc.dram_tensor("bq", [128, GROUP], f32, kind="ExternalInput").ap()
    onesk = nc.dram_tensor("onesk", [128, 128], bf16, kind="ExternalInput").ap()
    out = nc.dram_tensor("out", [S, HIDDEN], bf16, kind="ExternalOutput").ap()

    with tile.TileContext(nc) as tc:
        with (
            tc.tile_pool(name="const", bufs=1) as constp,
            tc.tile_pool(name="wts", bufs=1) as wtsp,
            tc.tile_pool(name="xb", bufs=1) as xbp,
            tc.tile_pool(name="qkv", bufs=1) as qkvp,
            tc.tile_pool(name="ptbuf", bufs=2) as ptp,
            tc.tile_pool(name="rbuf", bufs=2) as rp,
            tc.tile_pool(name="dens", bufs=2) as densp,
            tc.tile_pool(name="ytbuf", bufs=8) as ytp,
            tc.tile_pool(name="outbuf", bufs=2) as outp,
        ):
            # ---- constants that need no DMA (feed the PE warmup) ----
            ident = constp.tile([128, 128], f32, name="ident")
            make_identity(nc, ident[:, :])
            ident_bf = constp.tile([128, 128], bf16, name="ident_bf")
            nc.vector.tensor_copy(ident_bf[:, :], ident[:, :])
            wrhs = constp.tile([128, BAND], bf16, name="wrhs")
            nc.vector.memset(wrhs[:, :], 0.0)

            # ---- DMAs in consumption order ----
            onesk_t = constp.tile([128, 128], bf16, name="onesk_t")
            nc.sync.dma_start(out=onesk_t[:, :], in_=onesk)
            bq_t = constp.tile([128, GROUP], f32, name="bq_t")
            nc.sync.dma_start(out=bq_t[:, :], in_=bq)

            # interleave wk quarters with x band-0 quarters so the first
            # K-projection matmuls start as soon as the first pair lands
            wk_t = wtsp.tile([128, NCH, 128], bf16, name="wk_t")
            b0q = []
            for c4 in range(4):
                nc.sync.dma_start(
                    out=wk_t[:, 4 * c4:4 * c4 + 4, :], in_=wk[:, 4 * c4:4 * c4 + 4, :]
                )
                t = xbp.tile([128, 4, BAND], bf16, name=f"b0q{c4}", tag="band0q",
                             bufs=4)
                nc.sync.dma_start(out=t[:, :, :], in_=xT[0, :, 4 * c4:4 * c4 + 4, :])
                b0q.append(t)

            wv_t = wtsp.tile([128, NCH, 128], bf16, name="wv_t")
            nc.sync.dma_start(out=wv_t[:, :, :], in_=wv)
            wq_t = []
            for g in range(GROUP):
                t = wtsp.tile([128, NCH, 128], bf16, name=f"wq_t{g}", tag=f"wq{g}")
                nc.sync.dma_start(out=t[:, :, :], in_=wq[g])
                wq_t.append(t)

            bands = [None] * NBAND
            for bd in range(1, NBAND):
                bands[bd] = xbp.tile(
                    [128, NCH, BAND], bf16, name=f"band{bd}", tag=f"band{bd}"
                )
                nc.sync.dma_start(out=bands[bd][:, :, :], in_=xT[bd])

            wo_t = []
            for g in range(GROUP):
                t = wtsp.tile([128, HIDDEN], bf16, name=f"wo_t{g}", tag=f"wo{g}")
                nc.sync.dma_start(out=t[:, :], in_=wo[g])
                wo_t.append(t)

            # ---- persistent activations ----
            qt_t = []
            for g in range(GROUP):
                t = qkvp.tile([128, S], bf16, name=f"qt{g}", tag=f"qt{g}")
                qt_t.append(t)
            kt_t = qkvp.tile([128, S], bf16, name="kt_t")
            v_t = qkvp.tile([128, S], bf16, name="v_t")
            vt_b = qkvp.tile([128, S], bf16, name="vt_b")

            def bch(bd, c):
                if bd == 0:
                    return b0q[c // 4][:, c % 4, :]
                return bands[bd][:, c, :]

            # ====== phase 1: K/V all bands + Q band 0 (Q bands 1-3 are
            # deferred into the attention passes as PE filler) ======
            with tc.tile_pool(name="psA", bufs=1, space="PSUM") as psA:
                # PE warmup: no data dependencies; fills the DMA startup
                # hole and gets HAM past the cold window
                pwarm = psA.tile([128, BAND], f32, name="pwarm", tag="warm")
                for _ in range(NWARM):
                    nc.tensor.matmul(
                        out=pwarm[:, :], lhsT=ident_bf[:, :], rhs=wrhs[:, :],
                        start=True, stop=True,
                    )

                for bd in range(NBAND):
                    bsl = slice(bd * BAND, (bd + 1) * BAND)

                    # K^T accumulation
                    pk = psA.tile([128, BAND], f32, name="pk", tag="pacc", bufs=3)
                    for c in range(NCH):
                        nc.tensor.matmul(
                            out=pk[:, :],
                            lhsT=wk_t[:, c, :],
                            rhs=bch(bd, c),
                            start=(c == 0), stop=(c == NCH - 1),
                        )
                    nc.scalar.activation(kt_t[:, bsl], pk[:, :], COPY)

                    # V^T accumulation (bf16, transposed to V per 128-block)
                    pv = psA.tile([128, BAND], f32, name="pv", tag="pacc", bufs=3)
                    for c in range(NCH):
                        nc.tensor.matmul(
                            out=pv[:, :],
                            lhsT=wv_t[:, c, :],
                            rhs=bch(bd, c),
                            start=(c == 0), stop=(c == NCH - 1),
                        )
                    nc.scalar.activation(vt_b[:, bsl], pv[:, :], COPY)

                    # Q^T per local head, band 0 only
                    if bd == 0:
                        for g in range(GROUP):
                            pq = psA.tile([128, BAND], f32, name="pq", tag="pacc",
                                          bufs=3)
                            for c in range(NCH):
                                nc.tensor.matmul(
                                    out=pq[:, :],
                                    lhsT=wq_t[g][:, c, :],
                                    rhs=bch(0, c),
                                    start=(c == 0), stop=(c == NCH - 1),
                                )
                            nc.scalar.activation(
                                qt_t[g][:, bsl], pq[:, :], IDENT,
                                bias=bq_t[:, g:g + 1],
                            )

                    # transpose V^T band -> V (bf16 transpose, 1 cyc/row)
                    for t in range(BAND // 128):
                        sk = bd * (BAND // 128) + t
                        pt = psA.tile([128, 128], bf16, name="ptr", tag="ptr", bufs=2)
                        nc.tensor.transpose(
                            pt[:, :], vt_b[:, sk * 128:(sk + 1) * 128],
                            ident_bf[:, :]
                        )
                        nc.scalar.activation(
                            v_t[:, sk * 128:(sk + 1) * 128], pt[:, :], COPY
                        )

            # ====== phase 2: attention w/ interleaved filler + outproj ======
            # psB tags: sc x3 + py(yacc) x2 + aux x3 = 8 banks; pden shares
            # the aux rotation (it is live only ~1us inside the deferred
            # flush, and the per-pass aux call counts keep reuse distances
            # of >=9 sk for every rotation slot)
            with tc.tile_pool(name="psB", bufs=1, space="PSUM") as psB:
                yt_all = {}
                pending = [None]  # deferred den+normalize closure

                def flush_pending():
                    if pending[0] is not None:
                        pending[0]()
                        pending[0] = None

                def aux_tile(name):
                    return psB.tile([128, BAND], f32, name=name, tag="aux", bufs=3)

                def attn_group(qt, g, fillers=(), tail_fillers=()):
                    """One head-group of attention over qtile qt.

                    fillers: callbacks fill(sk) issuing exactly one PE matmul
                    each, called once per sk in 0..NSK-1 (16 calls); used for
                    deferred Q-band projections or interleaved out-projection
                    chunks. tail_fillers run after the trailing AVs.
                    """
                    qsl = slice(qt * QTILE, (qt + 1) * QTILE)
                    py = psB.tile([128, QTILE], f32, name="py", tag="yacc", bufs=2)
                    pt_all = ptp.tile([128, NSK * QTILE], bf16, name="pt", tag="pt")

                    def pt(a):
                        return pt_all[:, a * QTILE:(a + 1) * QTILE]

                    # DVE accumulation chain in bf16 for the softmax denominator
                    R = rp.tile([128, QTILE], bf16, name="racc", tag="racc")

                    for sk in range(NSK):
                        ps = psB.tile([128, QTILE], f32, name="ps", tag="sc", bufs=3)
                        nc.tensor.matmul(
                            out=ps[:, :],
                            lhsT=kt_t[:, sk * 128:(sk + 1) * 128],
                            rhs=qt_t[g][:, qsl],
                            start=True, stop=True,
                        )
                        nc.scalar.activation(pt(sk), ps[:, :], EXP, scale=SCALE)
                        if sk == 1:
                            nc.vector.tensor_add(R[:, :], pt(0), pt(1))
                        elif sk >= 2:
                            nc.vector.tensor_add(R[:, :], R[:, :], pt(sk))
                        if sk >= 2:
                            # AV trails scores by two sk so exp latency and
                            # jitter stay hidden
                            psk = sk - 2
                            nc.tensor.matmul(
                                out=py[:, :],
                                lhsT=v_t[:, psk * 128:(psk + 1) * 128],
                                rhs=pt(psk),
                                start=(psk == 0), stop=False,
                            )
                        for fill in fillers:
                            fill(sk)
                        if sk == 2:
                            flush_pending()
                    for psk in (NSK - 2, NSK - 1):
                        nc.tensor.matmul(
                            out=py[:, :],
                            lhsT=v_t[:, psk * 128:(psk + 1) * 128],
                            rhs=pt(psk),
                            start=False, stop=(psk == NSK - 1),
                        )
                    for tf in tail_fillers:
                        tf()

                    def normalize(qt=qt, g=g, py=py, R=R):
                        # all-ones stationary: out[i, q] = sum_p R[p, q] —
                        # partition-reduces AND replicates den on all 128
                        # partitions in one matmul
                        pden = aux_tile("pden")
                        nc.tensor.matmul(
                            out=pden[:, :],
                            lhsT=onesk_t[:, :],
                            rhs=R[:, :],
                            start=True, stop=True,
                        )
                        rb = densp.tile([128, QTILE], f32, name="rb", tag="rb")
                        nc.vector.reciprocal_approx_fast(rb[:, :], pden[:, :])
                        yt = ytp.tile([128, QTILE], bf16, name="yt", tag="yt")
                        nc.vector.tensor_mul(yt[:, :], py[:, :], rb[:, :])
                        yt_all[(qt, g)] = yt

                    pending[0] = normalize

                def make_qproj_filler(bd, g):
                    """Deferred Q-projection of (band bd, head g): one
                    contraction-chunk matmul per sk into an aux psum bank,
                    evacuated with the bias add at the end of the pass."""
                    qacc = aux_tile(f"qacc{bd}")
                    bsl = slice(bd * BAND, (bd + 1) * BAND)

                    def fill(sk):
                        nc.tensor.matmul(
                            out=qacc[:, :],
                            lhsT=wq_t[g][:, sk, :],
                            rhs=bands[bd][:, sk, :],
                            start=(sk == 0), stop=(sk == NCH - 1),
                        )

                    def evac():
                        nc.scalar.activation(
                            qt_t[g][:, bsl], qacc[:, :], IDENT,
                            bias=bq_t[:, g:g + 1],
                        )

                    return fill, evac

                def make_outproj_filler(proj_qt, i):
                    """Out-projection block (proj_qt, i): j-chunk sk//4 is
                    accumulated over the 4 heads during sk%4, then copied
                    out (DVE) and DMA'd per 512-column chunk."""
                    outs = outp.tile([128, HIDDEN], bf16, name="outs", tag="outs")
                    state = {}

                    def fill(sk):
                        j, gp = sk // 4, sk % 4
                        if gp == 0:
                            state["po"] = aux_tile("po")
                        po = state["po"]
                        nc.tensor.matmul(
                            out=po[:, :],
                            lhsT=yt_all[(proj_qt, gp)][:, i * 128:(i + 1) * 128],
                            rhs=wo_t[gp][:, j * 512:(j + 1) * 512],
                            start=(gp == 0), stop=(gp == GROUP - 1),
                        )
                        if gp == GROUP - 1:
                            csl = slice(j * 512, (j + 1) * 512)
                            nc.vector.tensor_copy(outs[:, csl], po[:, :])
                            r0 = proj_qt * QTILE + i * 128
                            nc.sync.dma_start(
                                out=out[r0:r0 + 128, j * 512:(j + 1) * 512],
                                in_=outs[:, csl],
                            )

                    return fill

                # qt0: + Q-projections of bands 1 and 2 (2 filler MMs/sk)
                for g in range(GROUP):
                    f1, e1 = make_qproj_filler(1, g)
                    f2, e2 = make_qproj_filler(2, g)
                    attn_group(0, g, fillers=(f1, f2), tail_fillers=(e1, e2))
                # qt1: + Q-projection of band 3
                for g in range(GROUP):
                    f3, e3 = make_qproj_filler(3, g)
                    attn_group(1, g, fillers=(f3,), tail_fillers=(e3,))
                # qt2/qt3: + out-projection of qt0/qt1 (i = g)
                for g in range(GROUP):
                    attn_group(2, g, fillers=(make_outproj_filler(0, g),))
                for g in range(GROUP):
                    attn_group(3, g, fillers=(make_outproj_filler(1, g),))
                flush_pending()

                # tail: out-projection of qt2 and qt3, pure PE
                for proj_qt in (2, 3):
                    for i in range(QTILE // 128):
                        outs = outp.tile(
                            [128, HIDDEN], bf16, name="outs", tag="outs"
                        )
                        for j in range(4):
                            po = aux_tile("po")
                            for gp in range(GROUP):
                                nc.tensor.matmul(
                                    out=po[:, :],
                                    lhsT=yt_all[(proj_qt, gp)][
                                        :, i * 128:(i + 1) * 128],
                                    rhs=wo_t[gp][:, j * 512:(j + 1) * 512],
                                    start=(gp == 0), stop=(gp == GROUP - 1),
                                )
                            csl = slice(j * 512, (j + 1) * 512)
                            nc.vector.tensor_copy(outs[:, csl], po[:, :])
                            r0 = proj_qt * QTILE + i * 128
                            nc.sync.dma_start(
                                out=out[r0:r0 + 128, j * 512:(j + 1) * 512],
                                in_=outs[:, csl],
                            )

    nc.finalize()
    return nc


def _get_nc():
    if "nc" not in _CACHE:
        _CACHE["nc"] = _build()
    return _CACHE["nc"]


def kernel(x, Wq, bq, Wk, bk, Wv, bv, Wo, bo):
    global LAST_RESULTS
    from concourse.bass_utils import run_bass_kernel_spmd

    bf = ml_dtypes.bfloat16
    x = np.asarray(x, np.float32)
    Wq = np.asarray(Wq, np.float32)
    Wk = np.asarray(Wk, np.float32)
    Wv = np.asarray(Wv, np.float32)
    Wo = np.asarray(Wo, np.float32)
    bq = np.asarray(bq, np.float32)
    bv = np.asarray(bv, np.float32)
    bo = np.asarray(bo, np.float32)

    nc = _get_nc()

    onesk_np = np.ones((128, 128), bf)

    in_maps = []
    for c in range(8):
        b, h = divmod(c, NKV)
        xTb = x[b].T  # [HIDDEN, S]
        xTh = np.ascontiguousarray(
            xTb.reshape(NCH, 128, NBAND, BAND).transpose(2, 1, 0, 3)
        ).astype(bf)
        # wq[g]: [128, NCH, 128] per local head
        wqh = np.ascontiguousarray(
            Wq[:, h * 512:(h + 1) * 512]
            .reshape(NCH, 128, GROUP, 128).transpose(2, 1, 0, 3)
        ).astype(bf)
        wkh = np.ascontiguousarray(
            Wk[:, h * 128:(h + 1) * 128].reshape(NCH, 128, 128).transpose(1, 0, 2)
        ).astype(bf)
        wvh = np.ascontiguousarray(
            Wv[:, h * 128:(h + 1) * 128].reshape(NCH, 128, 128).transpose(1, 0, 2)
        ).astype(bf)
        woh = np.ascontiguousarray(
            Wo[h * 512:(h + 1) * 512, :].reshape(GROUP, 128, HIDDEN)
        ).astype(bf)
        bqh = np.ascontiguousarray(
            bq[h * 512:(h + 1) * 512].reshape(GROUP, 128).T
        )
        in_maps.append({
            "xT": xTh, "wq": wqh, "wk": wkh, "wv": wvh, "wo": woh,
            "bq": bqh, "onesk": onesk_np,
        })

    res = run_bass_kernel_spmd(
        nc, in_maps, list(range(8)), trace=TRACE, tmpdir=TMPDIR
    )
    LAST_RESULTS = res

    # host-side constant bias: (bv repeated per head group) @ Wo + bo
    bv_rep = np.broadcast_to(
        bv.reshape(NKV, 1, D), (NKV, GROUP, D)
    ).reshape(HIDDEN)
    bias_row = bv_rep @ Wo + bo  # [HIDDEN]

    out = np.empty((B, S, HIDDEN), np.float32)
    for b in range(B):
        acc = res.results[b * NKV + 0]["out"].astype(np.float32)
        for h in range(1, NKV):
            acc = acc + res.results[b * NKV + h]["out"].astype(np.float32)
        out[b] = acc + bias_row
    return out
